# revision 5
# baseline (speedup 1.0000x reference)
"""Trainium2 Bass kernel for nn_DoubleResNetUpDown (B=4194304, L=30, F=5).

Pure data-parallel over 8 NeuronCores (batch axis split; tiny weights
replicated). Per core, rows form 1024 group-columns of 512 rows each,
processed in waves of 3 chunks (24 groups x 512 cols). State lives in
one 4-bank PSUM tile per wave (fp32):
  C2 of chunk c: CC[0:120, 512c:512c+512]   (partition 5g+f; row 120 = 1.0)
  C1 (all 3):    CC[0:88,  1536:2048]       (chunk c at rows 32c..32c+23)
Per layer: relu(C2||ones) -> SBUF f32r (DVE/ACT split) -> PE mm1
(K=121, M=88, zero-padded block-diag w1 sheet + bias row) accumulates
true c1 into C1; relu(C1) -> f32r -> PE mm2 (K=24, M=120) accumulates
into C2. fp32r matmuls run at full PE rate for N=512 and keep ~13
mantissa bits, so end-to-end error is ~1e-4 relative. Inputs enter via
bf16 hi/lo identity matmuls (exact to ~2^-16).
"""
import numpy as np
import ml_dtypes
from contextlib import ExitStack

import concourse.bacc as bacc
import concourse.mybir as mybir
import concourse.tile as tile
from concourse.bass_utils import run_bass_kernel_spmd

f32 = mybir.dt.float32
f32r = mybir.dt.float32r
bf16 = mybir.dt.bfloat16
RELU = mybir.ActivationFunctionType.Relu
IDENT = mybir.ActivationFunctionType.Identity

N_CORES = 8
B_TOTAL = 4194304
L = 30
F = 5
NCOL = 512
GFULL = 24
SHEETW = 152            # per-layer w1 sheet width (block at cols 64..87)
ROWS_CORE = B_TOTAL // N_CORES
N_GC = ROWS_CORE // NCOL
SPLIT = 996             # relu2 DVE/ACT column split (of 1536)
MAXCHUNK = 3


def _wave_plan(n_gc):
    waves, gc = [], 0
    while gc < n_gc:
        rem = n_gc - gc
        wave = []
        while rem > 0 and len(wave) < MAXCHUNK:
            g = min(GFULL, rem)
            wave.append((gc, g))
            gc += g
            rem -= g
        waves.append(wave)
    return waves


def _emit_wave(nc, pools, wave, dram):
    (ccps, x2p, x1p, rhp, y1p, y2p, consts) = pools
    w1sb, w2sb, identt, selt = consts
    nchunk = len(wave)
    gc0w = wave[0][0]
    gtot = sum(g for _, g in wave)
    ctot = nchunk * NCOL

    CC = ccps.tile([128, 2048], f32, tag="cc")
    C1 = CC[:, 1536:2048]

    # ---- input staging ----
    x1h_t = x1p.tile([gtot, NCOL], bf16, tag="x1h")
    x1l_t = x1p.tile([gtot, NCOL], bf16, tag="x1l")
    nc.sync.dma_start(x1h_t[:], dram["x1h"][gc0w:gc0w + gtot, :])
    nc.sync.dma_start(x1l_t[:], dram["x1l"][gc0w:gc0w + gtot, :])
    x2ts = []
    for c, (gc0, G) in enumerate(wave):
        P2 = 5 * G
        x2h_t = x2p.tile([121, NCOL], bf16, tag=f"x2h{c}")
        x2l_t = x2p.tile([120, NCOL], bf16, tag=f"x2l{c}")
        nc.sync.dma_start(x2h_t[0:P2, :],
                          dram["x2h"][gc0:gc0 + G].rearrange("g f n -> (g f) n"))
        nc.sync.dma_start(x2h_t[120:121, :], dram["ones"][0:1, :])
        nc.sync.dma_start(x2l_t[0:P2, :],
                          dram["x2l"][gc0:gc0 + G].rearrange("g f n -> (g f) n"))
        x2ts.append((x2h_t, x2l_t, P2))

    # ---- state init ----
    # C1 bank: one start=True matmul clears the bank, then accumulate lo.
    nc.tensor.matmul(C1[0:88, :], selt[0:gtot, 0:88], x1h_t[:],
                     start=True, stop=False)
    nc.tensor.matmul(C1[0:88, :], selt[0:gtot, 0:88], x1l_t[:],
                     start=False, stop=False)
    # C2: each chunk owns a whole bank; row 120 becomes the constant 1.0 row.
    for c, (gc0, G) in enumerate(wave):
        x2h_t, x2l_t, P2 = x2ts[c]
        cs = c * NCOL
        nc.tensor.matmul(CC[0:121, cs:cs + NCOL], identt[0:121, 0:121],
                         x2h_t[:], start=True, stop=False)
        nc.tensor.matmul(CC[0:P2, cs:cs + NCOL], identt[0:P2, 0:P2],
                         x2l_t[0:P2, :], start=False, stop=False)

    # ---- 30-layer loop ----
    for i in range(L):
        RA = rhp.tile([121, 2048], f32r, tag="ra")
        # relu2 over C2-part (incl. ones row), split DVE/ACT
        s = min(SPLIT, ctot)
        nc.vector.tensor_scalar_max(RA[0:121, 0:s], CC[0:121, 0:s], 0.0)
        if s < ctot:
            nc.scalar.activation(RA[0:121, s:ctot], CC[0:121, s:ctot],
                                 RELU, bias=0.0)
        # mm1 per chunk: C1 += sheet_view.T @ r2  (M=88, zero-padded cols)
        for c in range(nchunk):
            cs = c * NCOL
            off = i * SHEETW + (64 - 32 * c)
            nc.tensor.matmul(C1[0:88, :], w1sb[0:121, off:off + 88],
                             RA[0:121, cs:cs + NCOL],
                             start=False,
                             stop=(i == L - 1 and c == nchunk - 1))
        # relu1 (bias already accumulated via ones row)
        nc.scalar.activation(RA[0:88, 1536:2048], C1[0:88, :], RELU, bias=0.0)
        # mm2 per chunk: C2 += w2_block.T @ r1
        for c, (gc0, G) in enumerate(wave):
            cs = c * NCOL
            nc.tensor.matmul(CC[0:5 * G, cs:cs + NCOL],
                             w2sb[32 * c:32 * c + G, i * 120:i * 120 + 5 * G],
                             RA[32 * c:32 * c + G, 1536:2048],
                             start=False, stop=(i == L - 1))

    # ---- drain ----
    y1t = y1p.tile([88, NCOL], f32, tag="y1")
    nc.scalar.activation(y1t[0:88, :], C1[0:88, :], IDENT, bias=0.0)
    y2t = y2p.tile([120, 1536], f32, tag="y2")
    nc.vector.tensor_copy(y2t[0:120, 0:ctot], CC[0:120, 0:ctot])
    for c, (gc0, G) in enumerate(wave):
        cs = c * NCOL
        nc.sync.dma_start(dram["y1"][gc0:gc0 + G, :],
                          y1t[32 * c:32 * c + G, :])
        nc.sync.dma_start(dram["y2"][gc0:gc0 + G].rearrange("g f n -> (g f) n"),
                          y2t[0:5 * G, cs:cs + NCOL])


def build(n_gc=N_GC):
    nc = bacc.Bacc("TRN2", target_bir_lowering=False, debug=False)
    dram = {
        "x1h": nc.dram_tensor("x1h", [n_gc, NCOL], bf16, kind="ExternalInput").ap(),
        "x1l": nc.dram_tensor("x1l", [n_gc, NCOL], bf16, kind="ExternalInput").ap(),
        "x2h": nc.dram_tensor("x2h", [n_gc, F, NCOL], bf16, kind="ExternalInput").ap(),
        "x2l": nc.dram_tensor("x2l", [n_gc, F, NCOL], bf16, kind="ExternalInput").ap(),
        "w1": nc.dram_tensor("w1", [121, L * SHEETW], f32, kind="ExternalInput").ap(),
        "w2": nc.dram_tensor("w2", [88, L * 120], f32, kind="ExternalInput").ap(),
        "ident": nc.dram_tensor("ident", [121, 121], bf16, kind="ExternalInput").ap(),
        "sel": nc.dram_tensor("sel", [72, 88], bf16, kind="ExternalInput").ap(),
        "ones": nc.dram_tensor("ones", [1, NCOL], bf16, kind="ExternalInput").ap(),
        "y1": nc.dram_tensor("y1", [n_gc, NCOL], f32, kind="ExternalOutput").ap(),
        "y2": nc.dram_tensor("y2", [n_gc, F, NCOL], f32, kind="ExternalOutput").ap(),
    }

    with tile.TileContext(nc) as tc:
        with ExitStack() as ctx:
            cp = ctx.enter_context(tc.tile_pool(name="consts", bufs=1))
            ccps = ctx.enter_context(tc.tile_pool(name="ccps", bufs=2, space="PSUM"))
            x2p = ctx.enter_context(tc.tile_pool(name="x2p", bufs=2))
            x1p = ctx.enter_context(tc.tile_pool(name="x1p", bufs=2))
            rhp = ctx.enter_context(tc.tile_pool(name="rhp", bufs=3))
            y1p = ctx.enter_context(tc.tile_pool(name="y1p", bufs=2))
            y2p = ctx.enter_context(tc.tile_pool(name="y2p", bufs=2))

            w1st = cp.tile([121, L * SHEETW], f32)
            w2st = cp.tile([88, L * 120], f32)
            nc.sync.dma_start(w1st[:], dram["w1"][:])
            nc.sync.dma_start(w2st[:], dram["w2"][:])
            w1sb = cp.tile([121, L * SHEETW], f32r)
            w2sb = cp.tile([88, L * 120], f32r)
            nc.vector.tensor_copy(w1sb[:], w1st[:])
            nc.vector.tensor_copy(w2sb[:], w2st[:])
            identt = cp.tile([121, 121], bf16)
            selt = cp.tile([72, 88], bf16)
            nc.sync.dma_start(identt[:], dram["ident"][:])
            nc.sync.dma_start(selt[:], dram["sel"][:])

            consts = (w1sb, w2sb, identt, selt)
            pools = (ccps, x2p, x1p, rhp, y1p, y2p, consts)
            for wave in _wave_plan(n_gc):
                _emit_wave(nc, pools, wave, dram)

    nc.compile()
    return nc


def _bf(x):
    return np.asarray(x, dtype=ml_dtypes.bfloat16)


def _prep_consts(W1, b1, W2):
    # w1 sheets: [121, L*SHEETW]; layer i block at cols 64..87:
    #   sheet[5g+f, 64+g] = W1[i,0,f];  sheet[120, 64+g] = b1[i]
    w1s = np.zeros((L, 121, SHEETW), np.float32)
    for g in range(GFULL):
        w1s[:, 5 * g:5 * g + 5, 64 + g] = W1[:, 0, :]
        w1s[:, 120, 64 + g] = b1[:, 0]
    w1_dev = np.ascontiguousarray(w1s.transpose(1, 0, 2).reshape(121, L * SHEETW))
    # w2: [88, L*120]; rows 32c+g, cols i*120 + 5g+f = W2[i, f, 0]
    w2s = np.zeros((L, 88, 120), np.float32)
    for c in range(3):
        for g in range(GFULL):
            w2s[:, 32 * c + g, 5 * g:5 * g + 5] = W2[:, :, 0]
    w2_dev = np.ascontiguousarray(w2s.transpose(1, 0, 2).reshape(88, L * 120))
    ident_dev = _bf(np.eye(121, dtype=np.float32))
    sel = np.zeros((72, 88), np.float32)
    for c in range(3):
        for g in range(GFULL):
            sel[GFULL * c + g, 32 * c + g] = 1.0
    ones_dev = _bf(np.ones((1, NCOL), np.float32))
    return w1_dev, w2_dev, ident_dev, _bf(sel), ones_dev


_CACHE = {}
_last_in_maps = None


def _get_nc(n_gc):
    if n_gc not in _CACHE:
        _CACHE[n_gc] = build(n_gc)
    return _CACHE[n_gc]


def run(x1, x2, W1, b1, W2, trace=False, n_cores=N_CORES):
    rows_core = x1.shape[0] // n_cores
    n_gc = rows_core // NCOL
    x1 = np.asarray(x1, np.float32)
    x2 = np.asarray(x2, np.float32)
    w1_dev, w2_dev, ident_dev, sel_dev, ones_dev = _prep_consts(
        np.asarray(W1, np.float32), np.asarray(b1, np.float32),
        np.asarray(W2, np.float32))

    in_maps = []
    for k in range(n_cores):
        sl = slice(k * rows_core, (k + 1) * rows_core)
        x1c = x1[sl, 0].reshape(n_gc, NCOL)
        x2c = np.ascontiguousarray(
            x2[sl].reshape(n_gc, NCOL, F).transpose(0, 2, 1))
        x1h = _bf(x1c)
        x1l = _bf(x1c - x1h.astype(np.float32))
        x2h = _bf(x2c)
        x2l = _bf(x2c - x2h.astype(np.float32))
        in_maps.append({
            "x1h": x1h, "x1l": x1l, "x2h": x2h, "x2l": x2l,
            "w1": w1_dev, "w2": w2_dev, "ident": ident_dev,
            "sel": sel_dev, "ones": ones_dev,
        })

    global _last_in_maps
    _last_in_maps = in_maps
    nc = _get_nc(n_gc)
    br = run_bass_kernel_spmd(nc, in_maps, core_ids=list(range(n_cores)),
                              trace=trace)
    y1 = np.concatenate([r["y1"].reshape(rows_core, 1) for r in br.results])
    y2 = np.concatenate([
        np.ascontiguousarray(r["y2"].transpose(0, 2, 1)).reshape(rows_core, F)
        for r in br.results])
    return (y1, y2), br


def kernel(x1, x2, W1, b1, W2):
    (y1, y2), _ = run(x1, x2, W1, b1, W2, trace=False)
    return (y1, y2)
